# revision 15
# baseline (speedup 1.0000x reference)
"""Trainium2 Bass kernel for CartNN minimal-NEAT forward pass.

Computes out = tanh(tanh(x @ w + b))[:, None] for x [16384, 4096] f32,
w [4096] f32, b [1] f32, data-parallel across 8 NeuronCores (2048 batch
rows per core).

Per-core plan (memory-bound; ~32 MiB of x per core):
  - w is DMA-broadcast once to a [128, 4096] SBUF tile, b to [128, 1].
  - x arrives as 16 tiles of [128 partitions, 4096] (contiguous 16 KiB
    per partition -> full DMA line rate).
  - One fused VectorE tensor_tensor_reduce per tile computes
    acc[p] = b + sum_k x[p,k]*w[k]  (mul + reduce-add in a single pass,
    ~2.9 us/tile, well under the ~5.9 us/tile DMA time).
  - Two ScalarE Tanh activations on the [128, 16] accumulator.
  - One 8 KiB DMA writes the [2048, 1] output slice.
"""

import numpy as np

import concourse.bacc as bacc
import concourse.mybir as mybir
from concourse.bass_utils import run_bass_kernel_spmd
from concourse.masks import make_identity
from concourse.tile import TileContext

N_CORES = 8
BATCH = 16384
IN_SIZE = 4096
P = 128
B_PER_CORE = BATCH // N_CORES  # 2048
N_TILES = B_PER_CORE // P  # 16

_NC_CACHE = None


def _build():
    nc = bacc.Bacc(
        "TRN2",
        target_bir_lowering=False,
        debug=False,
        num_devices=N_CORES,
    )
    x = nc.dram_tensor(
        "x", [B_PER_CORE, IN_SIZE], mybir.dt.float32, kind="ExternalInput"
    )
    w = nc.dram_tensor("w", [IN_SIZE], mybir.dt.float32, kind="ExternalInput")
    b = nc.dram_tensor("b", [1], mybir.dt.float32, kind="ExternalInput")
    y = nc.dram_tensor("y", [B_PER_CORE, 1], mybir.dt.float32, kind="ExternalOutput")

    xt = x.rearrange("(t p) k -> t p k", p=P)  # [16, 128, 4096]
    yT = y.rearrange("(t p) o -> t (p o)", p=P)  # [16, 128], 512B rows

    with TileContext(nc) as tc:
        with (
            tc.tile_pool(name="xpool", bufs=4) as xpool,
            tc.tile_pool(name="scratch", bufs=2) as spool,
            tc.tile_pool(name="consts", bufs=1) as cpool,
            tc.tile_pool(name="psum", bufs=1, space="PSUM") as ppool,
        ):
            # Broadcast w to all 128 partitions: two parallel stride-0 DRAM
            # broadcasts fill partitions [0:32] and [64:96], then two
            # parallel SBUF->SBUF copies double them to [32:64]/[96:128].
            # Serial depth 2 (~7 us, hidden under the x ramp-up). All on
            # the scalar engine's HWDGE ring: anything serial on the sync
            # ring head-of-line-blocks the x-load descriptor stream.
            w_PK = cpool.tile([P, IN_SIZE], mybir.dt.float32)
            for base in (0, 64):
                nc.scalar.dma_start(
                    out=w_PK[base : base + 32, :],
                    in_=w[None, :].to_broadcast((32, IN_SIZE)),
                )
            for base in (0, 64):
                nc.scalar.dma_start(
                    out=w_PK[base + 32 : base + 64, :],
                    in_=w_PK[base : base + 32, :],
                )
            b_P1 = cpool.tile([P, 1], mybir.dt.float32)
            nc.scalar.dma_start(out=b_P1[:], in_=b[None, :].to_broadcast((P, 1)))
            ident = cpool.tile([P, P], mybir.dt.float32)
            make_identity(nc, ident[:])
            acc_PT = cpool.tile([P, N_TILES], mybir.dt.float32)

            for t in range(N_TILES):
                x_PK = xpool.tile([P, IN_SIZE], mybir.dt.float32)
                nc.sync.dma_start(out=x_PK[:], in_=xt[t])
                # Fused dot product on VectorE: one pass reads x and w and
                # reduce-adds into acc[:, t]. Keeping all compute in a
                # single DVE op minimizes SBUF traffic so DMA can stream x
                # at the fabric ceiling (~433 GB/s measured).
                prod_PK = spool.tile([P, IN_SIZE], mybir.dt.float32)
                nc.vector.affine_mul_reduce(
                    out=prod_PK[:],
                    accum_out=acc_PT[:, t : t + 1],
                    in0=x_PK[:],
                    in1=w_PK[:],
                    scale=1.0,
                    bias=0.0,
                )

            # Transpose [128 rows, 16 tiles] -> [16, 128] on the (idle)
            # TensorE so the output DMA writes 512B-contiguous runs; the
            # partition-major layout cost a 12 us completion wait (2048
            # 4-byte descriptors).
            acc_T = ppool.tile([N_TILES, P], mybir.dt.float32)
            nc.tensor.transpose(acc_T[:], acc_PT[:], ident[:])
            y_TP = cpool.tile([N_TILES, P], mybir.dt.float32)
            nc.scalar.activation(
                y_TP[:],
                acc_T[:],
                mybir.ActivationFunctionType.Tanh,
                bias=b_P1[0:N_TILES, :],
            )
            nc.scalar.activation(y_TP[:], y_TP[:], mybir.ActivationFunctionType.Tanh)
            nc.sync.dma_start(out=yT, in_=y_TP[:])
    nc.compile()
    return nc


def _get_nc():
    global _NC_CACHE
    if _NC_CACHE is None:
        _NC_CACHE = _build()
    return _NC_CACHE


def _run(x, w, b, **spmd_kwargs):
    """Shard, execute on 8 cores, gather. Returns (out, BassKernelResults)."""
    x = np.ascontiguousarray(np.asarray(x, dtype=np.float32))
    w = np.ascontiguousarray(np.asarray(w, dtype=np.float32))
    b = np.ascontiguousarray(np.asarray(b, dtype=np.float32))
    assert x.shape == (BATCH, IN_SIZE), x.shape

    nc = _get_nc()
    in_maps = [
        {"x": x[c * B_PER_CORE : (c + 1) * B_PER_CORE], "w": w, "b": b}
        for c in range(N_CORES)
    ]
    res = run_bass_kernel_spmd(nc, in_maps, list(range(N_CORES)), **spmd_kwargs)
    out = np.concatenate(
        [np.asarray(res.results[c]["y"]) for c in range(N_CORES)], axis=0
    )
    return out.astype(np.float32, copy=False), res


def kernel(x, w, b):
    out, _ = _run(x, w, b)
    return out


# revision 17
# speedup vs baseline: 1.0565x; 1.0565x over previous
"""Trainium2 Bass kernel for CartNN minimal-NEAT forward pass.

Computes out = tanh(tanh(x @ w + b))[:, None] for x [16384, 4096] f32,
w [4096] f32, b [1] f32, data-parallel across 8 NeuronCores (2048 batch
rows per core).

Per-core plan (memory-bound; ~32 MiB of x per core):
  - w is DMA-broadcast once to a [128, 4096] SBUF tile, b to [128, 1].
  - x arrives as 16 tiles of [128 partitions, 4096] (contiguous 16 KiB
    per partition -> full DMA line rate).
  - One fused VectorE tensor_tensor_reduce per tile computes
    acc[p] = b + sum_k x[p,k]*w[k]  (mul + reduce-add in a single pass,
    ~2.9 us/tile, well under the ~5.9 us/tile DMA time).
  - Two ScalarE Tanh activations on the [128, 16] accumulator.
  - One 8 KiB DMA writes the [2048, 1] output slice.
"""

import numpy as np

import concourse.bacc as bacc
import concourse.mybir as mybir
from concourse.bass_utils import run_bass_kernel_spmd
from concourse.masks import make_identity
from concourse.tile import TileContext

N_CORES = 8
BATCH = 16384
IN_SIZE = 4096
P = 128
B_PER_CORE = BATCH // N_CORES  # 2048
N_TILES = B_PER_CORE // P  # 16

_NC_CACHE = None


def _build():
    nc = bacc.Bacc(
        "TRN2",
        target_bir_lowering=False,
        debug=False,
        num_devices=N_CORES,
    )
    x = nc.dram_tensor(
        "x", [B_PER_CORE, IN_SIZE], mybir.dt.float32, kind="ExternalInput"
    )
    w = nc.dram_tensor("w", [IN_SIZE], mybir.dt.float32, kind="ExternalInput")
    b = nc.dram_tensor("b", [1], mybir.dt.float32, kind="ExternalInput")
    y = nc.dram_tensor("y", [B_PER_CORE, 1], mybir.dt.float32, kind="ExternalOutput")

    xt = x.rearrange("(t p) k -> t p k", p=P)  # [16, 128, 4096]
    yT = y.rearrange("(t p) o -> t (p o)", p=P)  # [16, 128], 512B rows

    with TileContext(nc) as tc:
        with (
            tc.tile_pool(name="xpool", bufs=4) as xpool,
            tc.tile_pool(name="scratch", bufs=2) as spool,
            tc.tile_pool(name="consts", bufs=1) as cpool,
            tc.tile_pool(name="psum", bufs=1, space="PSUM") as ppool,
        ):
            # w arrives as one plain 16 KiB load (first on the sync ring,
            # single descriptor — stride-0 DRAM broadcast DMAs measurably
            # poison the whole x stream), then TensorE broadcasts it to all
            # 128 partitions chunk by chunk: ones[128,1] @ w[1,512] outer
            # products, copied PSUM->SBUF by the otherwise-idle ScalarE.
            w_1K = cpool.tile([1, IN_SIZE], mybir.dt.float32)
            nc.sync.dma_start(out=w_1K[:], in_=w[None, :])
            b_11 = cpool.tile([1, 1], mybir.dt.float32)
            nc.scalar.dma_start(out=b_11[:], in_=b[None, :])
            ones_1P = cpool.tile([1, P], mybir.dt.float32)
            nc.vector.memset(ones_1P[:], 1.0)
            w_PK = cpool.tile([P, IN_SIZE], mybir.dt.float32)
            NCHUNK = 512
            for c in range(IN_SIZE // NCHUNK):
                cs = slice(c * NCHUNK, (c + 1) * NCHUNK)
                w_psum = ppool.tile([P, NCHUNK], mybir.dt.float32, bufs=2)
                nc.tensor.matmul(w_psum[:], ones_1P[:], w_1K[0:1, cs])
                nc.scalar.copy(w_PK[:, cs], w_psum[:])
            b_psum = ppool.tile([N_TILES, 1], mybir.dt.float32)
            nc.tensor.matmul(b_psum[:], ones_1P[0:1, 0:N_TILES], b_11[:])
            b_T1 = cpool.tile([N_TILES, 1], mybir.dt.float32)
            nc.scalar.copy(b_T1[:], b_psum[:])
            ident = cpool.tile([P, P], mybir.dt.float32)
            make_identity(nc, ident[:])
            acc_a = cpool.tile([P, N_TILES], mybir.dt.float32)
            acc_b = cpool.tile([P, N_TILES], mybir.dt.float32)

            KH = IN_SIZE // 2
            for t in range(N_TILES):
                x_PK = xpool.tile([P, IN_SIZE], mybir.dt.float32)
                nc.sync.dma_start(out=x_PK[:], in_=xt[t])
                # Fused dot product on VectorE (single pass: mul + reduce),
                # split into two half-K ops so the first can start as soon
                # as the first half of the w broadcast is built (~11 us
                # earlier than waiting for all of w_PK).
                prod_PK = spool.tile([P, IN_SIZE], mybir.dt.float32)
                nc.vector.affine_mul_reduce(
                    out=prod_PK[:, 0:KH],
                    accum_out=acc_a[:, t : t + 1],
                    in0=x_PK[:, 0:KH],
                    in1=w_PK[:, 0:KH],
                    scale=1.0,
                    bias=0.0,
                )
                nc.vector.affine_mul_reduce(
                    out=prod_PK[:, KH:IN_SIZE],
                    accum_out=acc_b[:, t : t + 1],
                    in0=x_PK[:, KH:IN_SIZE],
                    in1=w_PK[:, KH:IN_SIZE],
                    scale=1.0,
                    bias=0.0,
                )
            acc_PT = cpool.tile([P, N_TILES], mybir.dt.float32)
            nc.vector.tensor_add(acc_PT[:], acc_a[:], acc_b[:])

            # Transpose [128 rows, 16 tiles] -> [16, 128] on the (idle)
            # TensorE so the output DMA writes 512B-contiguous runs; the
            # partition-major layout cost a 12 us completion wait (2048
            # 4-byte descriptors).
            acc_T = ppool.tile([N_TILES, P], mybir.dt.float32)
            nc.tensor.transpose(acc_T[:], acc_PT[:], ident[:])
            y_TP = cpool.tile([N_TILES, P], mybir.dt.float32)
            nc.scalar.activation(
                y_TP[:],
                acc_T[:],
                mybir.ActivationFunctionType.Tanh,
                bias=b_T1[:],
            )
            nc.scalar.activation(y_TP[:], y_TP[:], mybir.ActivationFunctionType.Tanh)
            nc.sync.dma_start(out=yT, in_=y_TP[:])
    nc.compile()
    return nc


def _get_nc():
    global _NC_CACHE
    if _NC_CACHE is None:
        _NC_CACHE = _build()
    return _NC_CACHE


def _run(x, w, b, **spmd_kwargs):
    """Shard, execute on 8 cores, gather. Returns (out, BassKernelResults)."""
    x = np.ascontiguousarray(np.asarray(x, dtype=np.float32))
    w = np.ascontiguousarray(np.asarray(w, dtype=np.float32))
    b = np.ascontiguousarray(np.asarray(b, dtype=np.float32))
    assert x.shape == (BATCH, IN_SIZE), x.shape

    nc = _get_nc()
    in_maps = [
        {"x": x[c * B_PER_CORE : (c + 1) * B_PER_CORE], "w": w, "b": b}
        for c in range(N_CORES)
    ]
    res = run_bass_kernel_spmd(nc, in_maps, list(range(N_CORES)), **spmd_kwargs)
    out = np.concatenate(
        [np.asarray(res.results[c]["y"]) for c in range(N_CORES)], axis=0
    )
    return out.astype(np.float32, copy=False), res


def kernel(x, w, b):
    out, _ = _run(x, w, b)
    return out


# revision 19
# speedup vs baseline: 1.2742x; 1.2061x over previous
"""Trainium2 Bass kernel for CartNN minimal-NEAT forward pass.

Computes out = tanh(tanh(x @ w + b))[:, None] for x [16384, 4096] f32,
w [4096] f32, b [1] f32, data-parallel across 8 NeuronCores (2048 batch
rows per core).

Per-core plan (memory-bound; ~32 MiB of x per core):
  - w is DMA-broadcast once to a [128, 4096] SBUF tile, b to [128, 1].
  - x arrives as 16 tiles of [128 partitions, 4096] (contiguous 16 KiB
    per partition -> full DMA line rate).
  - One fused VectorE tensor_tensor_reduce per tile computes
    acc[p] = b + sum_k x[p,k]*w[k]  (mul + reduce-add in a single pass,
    ~2.9 us/tile, well under the ~5.9 us/tile DMA time).
  - Two ScalarE Tanh activations on the [128, 16] accumulator.
  - One 8 KiB DMA writes the [2048, 1] output slice.
"""

import numpy as np

import concourse.bacc as bacc
import concourse.mybir as mybir
from concourse.bass_utils import run_bass_kernel_spmd
from concourse.masks import make_identity
from concourse.tile import TileContext

N_CORES = 8
BATCH = 16384
IN_SIZE = 4096
P = 128
B_PER_CORE = BATCH // N_CORES  # 2048
N_TILES = B_PER_CORE // P  # 16

_NC_CACHE = None


def _build():
    nc = bacc.Bacc(
        "TRN2",
        target_bir_lowering=False,
        debug=False,
        num_devices=N_CORES,
    )
    x = nc.dram_tensor(
        "x", [B_PER_CORE, IN_SIZE], mybir.dt.float32, kind="ExternalInput"
    )
    w = nc.dram_tensor("w", [IN_SIZE], mybir.dt.float32, kind="ExternalInput")
    b = nc.dram_tensor("b", [1], mybir.dt.float32, kind="ExternalInput")
    y = nc.dram_tensor("y", [B_PER_CORE, 1], mybir.dt.float32, kind="ExternalOutput")

    xt = x.rearrange("(t p) k -> t p k", p=P)  # [16, 128, 4096]
    yT = y.rearrange("(t p) o -> t (p o)", p=P)  # [16, 128], 512B rows

    with TileContext(nc) as tc:
        with (
            tc.tile_pool(name="xpool", bufs=8) as xpool,
            tc.tile_pool(name="scratch", bufs=1) as spool,
            tc.tile_pool(name="consts", bufs=1) as cpool,
            tc.tile_pool(name="psum", bufs=1, space="PSUM") as ppool,
        ):
            # w arrives as one plain 16 KiB load (first on the sync ring,
            # single descriptor — stride-0 DRAM broadcast DMAs measurably
            # poison the whole x stream), then TensorE broadcasts it to all
            # 128 partitions chunk by chunk: ones[128,1] @ w[1,512] outer
            # products, copied PSUM->SBUF by the otherwise-idle ScalarE.
            w_1K = cpool.tile([1, IN_SIZE], mybir.dt.float32)
            nc.sync.dma_start(out=w_1K[:], in_=w[None, :])
            b_11 = cpool.tile([1, 1], mybir.dt.float32)
            nc.scalar.dma_start(out=b_11[:], in_=b[None, :])
            ones_1P = cpool.tile([1, P], mybir.dt.float32)
            nc.vector.memset(ones_1P[:], 1.0)
            w_PK = cpool.tile([P, IN_SIZE], mybir.dt.float32)
            NCHUNK = 512
            for c in range(IN_SIZE // NCHUNK):
                cs = slice(c * NCHUNK, (c + 1) * NCHUNK)
                w_psum = ppool.tile([P, NCHUNK], mybir.dt.float32, bufs=2)
                nc.tensor.matmul(w_psum[:], ones_1P[:], w_1K[0:1, cs])
                nc.scalar.copy(w_PK[:, cs], w_psum[:])
            b_psum = ppool.tile([N_TILES, 1], mybir.dt.float32)
            nc.tensor.matmul(b_psum[:], ones_1P[0:1, 0:N_TILES], b_11[:])
            b_T1 = cpool.tile([N_TILES, 1], mybir.dt.float32)
            nc.scalar.copy(b_T1[:], b_psum[:])
            ident = cpool.tile([P, P], mybir.dt.float32)
            make_identity(nc, ident[:])

            # Per-tile dot product = 4 quarter-K fused mul+reduce DVE ops,
            # emitted with a staggered schedule (quarter q of tile t at
            # step t + 2q): quarter q only needs w[q*1024:(q+1)*1024], so
            # VectorE starts as soon as the first w chunk is broadcast
            # (~14 us) instead of waiting for all of w (~27 us), and never
            # stalls on a not-yet-broadcast chunk. The Tile scheduler keeps
            # same-engine program order, so the stagger must be explicit.
            NSPLIT = 4
            KQ = IN_SIZE // NSPLIT
            accs = []
            for q in range(NSPLIT):
                acc_q = cpool.tile(
                    [P, N_TILES], mybir.dt.float32, name=f"acc_{q}"
                )
                accs.append(acc_q)
            prod_PK = spool.tile([P, IN_SIZE], mybir.dt.float32)
            x_tiles = {}

            def emit_quarter(t, q):
                seg = slice(q * KQ, (q + 1) * KQ)
                nc.vector.affine_mul_reduce(
                    out=prod_PK[:, seg],
                    accum_out=accs[q][:, t : t + 1],
                    in0=x_tiles[t][:, seg],
                    in1=w_PK[:, seg],
                    scale=1.0,
                    bias=0.0,
                )

            STAGGER = 2
            for i in range(N_TILES + STAGGER * (NSPLIT - 1)):
                if i < N_TILES:
                    x_PK = xpool.tile([P, IN_SIZE], mybir.dt.float32)
                    nc.sync.dma_start(out=x_PK[:], in_=xt[i])
                    x_tiles[i] = x_PK
                    emit_quarter(i, 0)
                for q in range(1, NSPLIT):
                    t = i - STAGGER * q
                    if 0 <= t < N_TILES:
                        emit_quarter(t, q)

            acc_PT = cpool.tile([P, N_TILES], mybir.dt.float32)
            nc.vector.tensor_add(acc_PT[:], accs[0][:], accs[1][:])
            nc.vector.tensor_add(accs[2][:], accs[2][:], accs[3][:])
            nc.vector.tensor_add(acc_PT[:], acc_PT[:], accs[2][:])

            # Transpose [128 rows, 16 tiles] -> [16, 128] on the (idle)
            # TensorE so the output DMA writes 512B-contiguous runs; the
            # partition-major layout cost a 12 us completion wait (2048
            # 4-byte descriptors).
            acc_T = ppool.tile([N_TILES, P], mybir.dt.float32)
            nc.tensor.transpose(acc_T[:], acc_PT[:], ident[:])
            y_TP = cpool.tile([N_TILES, P], mybir.dt.float32)
            nc.scalar.activation(
                y_TP[:],
                acc_T[:],
                mybir.ActivationFunctionType.Tanh,
                bias=b_T1[:],
            )
            nc.scalar.activation(y_TP[:], y_TP[:], mybir.ActivationFunctionType.Tanh)
            nc.sync.dma_start(out=yT, in_=y_TP[:])
    nc.compile()
    return nc


def _get_nc():
    global _NC_CACHE
    if _NC_CACHE is None:
        _NC_CACHE = _build()
    return _NC_CACHE


def _run(x, w, b, **spmd_kwargs):
    """Shard, execute on 8 cores, gather. Returns (out, BassKernelResults)."""
    x = np.ascontiguousarray(np.asarray(x, dtype=np.float32))
    w = np.ascontiguousarray(np.asarray(w, dtype=np.float32))
    b = np.ascontiguousarray(np.asarray(b, dtype=np.float32))
    assert x.shape == (BATCH, IN_SIZE), x.shape

    nc = _get_nc()
    in_maps = [
        {"x": x[c * B_PER_CORE : (c + 1) * B_PER_CORE], "w": w, "b": b}
        for c in range(N_CORES)
    ]
    res = run_bass_kernel_spmd(nc, in_maps, list(range(N_CORES)), **spmd_kwargs)
    out = np.concatenate(
        [np.asarray(res.results[c]["y"]) for c in range(N_CORES)], axis=0
    )
    return out.astype(np.float32, copy=False), res


def kernel(x, w, b):
    out, _ = _run(x, w, b)
    return out


# revision 20
# speedup vs baseline: 1.2845x; 1.0081x over previous
"""Trainium2 Bass kernel for CartNN minimal-NEAT forward pass.

Computes out = tanh(tanh(x @ w + b))[:, None] for x [16384, 4096] f32,
w [4096] f32, b [1] f32, data-parallel across 8 NeuronCores (2048 batch
rows per core).

Per-core plan (memory-bound; ~32 MiB of x per core):
  - w is DMA-broadcast once to a [128, 4096] SBUF tile, b to [128, 1].
  - x arrives as 16 tiles of [128 partitions, 4096] (contiguous 16 KiB
    per partition -> full DMA line rate).
  - One fused VectorE tensor_tensor_reduce per tile computes
    acc[p] = b + sum_k x[p,k]*w[k]  (mul + reduce-add in a single pass,
    ~2.9 us/tile, well under the ~5.9 us/tile DMA time).
  - Two ScalarE Tanh activations on the [128, 16] accumulator.
  - One 8 KiB DMA writes the [2048, 1] output slice.
"""

import numpy as np

import concourse.bacc as bacc
import concourse.mybir as mybir
from concourse.bass_utils import run_bass_kernel_spmd
from concourse.masks import make_identity
from concourse.tile import TileContext

N_CORES = 8
BATCH = 16384
IN_SIZE = 4096
P = 128
B_PER_CORE = BATCH // N_CORES  # 2048
N_TILES = B_PER_CORE // P  # 16

_NC_CACHE = None


def _build():
    nc = bacc.Bacc(
        "TRN2",
        target_bir_lowering=False,
        debug=False,
        num_devices=N_CORES,
    )
    x = nc.dram_tensor(
        "x", [B_PER_CORE, IN_SIZE], mybir.dt.float32, kind="ExternalInput"
    )
    w = nc.dram_tensor("w", [IN_SIZE], mybir.dt.float32, kind="ExternalInput")
    b = nc.dram_tensor("b", [1], mybir.dt.float32, kind="ExternalInput")
    y = nc.dram_tensor("y", [B_PER_CORE, 1], mybir.dt.float32, kind="ExternalOutput")

    xt = x.rearrange("(t p) k -> t p k", p=P)  # [16, 128, 4096]
    yT = y.rearrange("(t p) o -> t (p o)", p=P)  # [16, 128], 512B rows

    with TileContext(nc) as tc:
        with (
            tc.tile_pool(name="xpool", bufs=8) as xpool,
            tc.tile_pool(name="scratch", bufs=1) as spool,
            tc.tile_pool(name="consts", bufs=1) as cpool,
            tc.tile_pool(name="psum", bufs=1, space="PSUM") as ppool,
        ):
            # w arrives as one plain 16 KiB load (first on the sync ring,
            # single descriptor — stride-0 DRAM broadcast DMAs measurably
            # poison the whole x stream), then TensorE broadcasts it to all
            # 128 partitions chunk by chunk: ones[128,1] @ w[1,512] outer
            # products, copied PSUM->SBUF by the otherwise-idle ScalarE.
            w_1K = cpool.tile([1, IN_SIZE], mybir.dt.float32)
            nc.sync.dma_start(out=w_1K[:], in_=w[None, :])
            b_11 = cpool.tile([1, 1], mybir.dt.float32)
            nc.scalar.dma_start(out=b_11[:], in_=b[None, :])
            ones_1P = cpool.tile([1, P], mybir.dt.float32)
            nc.vector.memset(ones_1P[:], 1.0)
            w_PK = cpool.tile([P, IN_SIZE], mybir.dt.float32)
            NCHUNK = 512
            for c in range(IN_SIZE // NCHUNK):
                cs = slice(c * NCHUNK, (c + 1) * NCHUNK)
                w_psum = ppool.tile([P, NCHUNK], mybir.dt.float32, bufs=2)
                nc.tensor.matmul(w_psum[:], ones_1P[:], w_1K[0:1, cs])
                nc.scalar.copy(w_PK[:, cs], w_psum[:])
            b_psum = ppool.tile([N_TILES, 1], mybir.dt.float32)
            nc.tensor.matmul(b_psum[:], ones_1P[0:1, 0:N_TILES], b_11[:])
            b_T1 = cpool.tile([N_TILES, 1], mybir.dt.float32)
            nc.scalar.copy(b_T1[:], b_psum[:])
            ident = cpool.tile([P, P], mybir.dt.float32)
            make_identity(nc, ident[:])

            # VectorE does one fused mul+reduce pass per tile. The first 4
            # tiles are split into quarter-K ops with a staggered emission
            # (quarter q of tile t at step t + 3q): quarter q only needs
            # w[q*1024:(q+1)*1024], so DVE starts as soon as the first w
            # chunk is broadcast (~14 us) instead of waiting for all of w
            # (~27 us). Later tiles use a single full-K op — less
            # per-instruction overhead once w is complete. The Tile
            # scheduler keeps same-engine program order, so the stagger
            # must be explicit.
            NSPLIT = 4
            NQT = 4  # tiles that use the quarter-split
            STAGGER = 3
            KQ = IN_SIZE // NSPLIT
            acc_PT = cpool.tile([P, N_TILES], mybir.dt.float32)
            accs_q = [
                cpool.tile([P, NQT], mybir.dt.float32, name=f"acc_{q}")
                for q in range(1, NSPLIT)
            ]
            prod_PK = spool.tile([P, IN_SIZE], mybir.dt.float32)
            x_tiles = {}

            def load_x(t):
                x_PK = xpool.tile([P, IN_SIZE], mybir.dt.float32)
                nc.sync.dma_start(out=x_PK[:], in_=xt[t])
                x_tiles[t] = x_PK

            def emit_quarter(t, q):
                seg = slice(q * KQ, (q + 1) * KQ)
                acc = acc_PT[:, t : t + 1] if q == 0 else accs_q[q - 1][:, t : t + 1]
                nc.vector.affine_mul_reduce(
                    out=prod_PK[:, seg],
                    accum_out=acc,
                    in0=x_tiles[t][:, seg],
                    in1=w_PK[:, seg],
                    scale=1.0,
                    bias=0.0,
                )

            for i in range(NQT + STAGGER * (NSPLIT - 1)):
                if i < NQT:
                    load_x(i)
                    emit_quarter(i, 0)
                for q in range(1, NSPLIT):
                    t = i - STAGGER * q
                    if 0 <= t < NQT:
                        emit_quarter(t, q)
            for t in range(NQT, N_TILES):
                load_x(t)
                nc.vector.affine_mul_reduce(
                    out=prod_PK[:],
                    accum_out=acc_PT[:, t : t + 1],
                    in0=x_tiles[t][:],
                    in1=w_PK[:],
                    scale=1.0,
                    bias=0.0,
                )
            for acc_q in accs_q:
                nc.vector.tensor_add(
                    acc_PT[:, 0:NQT], acc_PT[:, 0:NQT], acc_q[:]
                )

            # Transpose [128 rows, 16 tiles] -> [16, 128] on the (idle)
            # TensorE so the output DMA writes 512B-contiguous runs; the
            # partition-major layout cost a 12 us completion wait (2048
            # 4-byte descriptors).
            acc_T = ppool.tile([N_TILES, P], mybir.dt.float32)
            nc.tensor.transpose(acc_T[:], acc_PT[:], ident[:])
            y_TP = cpool.tile([N_TILES, P], mybir.dt.float32)
            nc.scalar.activation(
                y_TP[:],
                acc_T[:],
                mybir.ActivationFunctionType.Tanh,
                bias=b_T1[:],
            )
            nc.scalar.activation(y_TP[:], y_TP[:], mybir.ActivationFunctionType.Tanh)
            nc.sync.dma_start(out=yT, in_=y_TP[:])
    nc.compile()
    return nc


def _get_nc():
    global _NC_CACHE
    if _NC_CACHE is None:
        _NC_CACHE = _build()
    return _NC_CACHE


def _run(x, w, b, **spmd_kwargs):
    """Shard, execute on 8 cores, gather. Returns (out, BassKernelResults)."""
    x = np.ascontiguousarray(np.asarray(x, dtype=np.float32))
    w = np.ascontiguousarray(np.asarray(w, dtype=np.float32))
    b = np.ascontiguousarray(np.asarray(b, dtype=np.float32))
    assert x.shape == (BATCH, IN_SIZE), x.shape

    nc = _get_nc()
    in_maps = [
        {"x": x[c * B_PER_CORE : (c + 1) * B_PER_CORE], "w": w, "b": b}
        for c in range(N_CORES)
    ]
    res = run_bass_kernel_spmd(nc, in_maps, list(range(N_CORES)), **spmd_kwargs)
    out = np.concatenate(
        [np.asarray(res.results[c]["y"]) for c in range(N_CORES)], axis=0
    )
    return out.astype(np.float32, copy=False), res


def kernel(x, w, b):
    out, _ = _run(x, w, b)
    return out
